# revision 18
# baseline (speedup 1.0000x reference)
"""Trainium2 kernel for CompactBilinearLayer (count-sketch bilinear pooling).

Math: reference computes y = l2norm(signed_sqrt(sum_hw Re IFFT(FFT(x@M1)*FFT(x@M2)))).
Since M1/M2 are count-sketch matrices (one +-1 per row), FFT(x@M1) == x @ A1 with
A1[c,k] = s1[c] * exp(-2pi i h1[c] k / P) — a dense [512, K] matrix computable on the
host from M1 in O(C*K). The IFFT is linear, so the spatial sum moves before it.
Hermitian symmetry means only k = 0..4096 are needed.  Per core (4 batch elements,
784 spatial positions — fully batch-local, no collectives):
  A: projections r1,i1,i2,r2 = x @ A-planes, single fp16 matmul each (rel-err
     budget 2e-2 allows it; measured ~1.2e-3 end to end)
  B: S[k,b] = sum_t P1*P2: ScalarE drains PSUM planes to SBUF fp16, then DVE
     tensor_tensor_reduce chains (fused multiply+reduce, fp32 accum) produce
     S_re = sum r1r2 - i1i2 and S_im = sum r1i2 + i1r2 directly
  C: IFFT via two-step factorization p=64q+s: Gauss 3-mult twiddle
     t1=cphi*S_re, t2=sphi*S_im, t3=(cphi+sphi)*(S_re+S_im), then
     psy = (cosa-nsina)@t1 - (cosa+nsina)@t2 + nsina@t3  (128-DFT matmuls)
  D: signed sqrt + per-batch L2 norm + store
"""
import numpy as np

P = 8192
C = 512
FT = 33            # frequency tiles of 128 -> 4224 slots >= 4097
NSLOT = FT * 128
NCORES = 8
BPC = 4            # batch elems per core
HW = 196           # spatial positions per batch elem
T = BPC * HW       # 784 positions per core
B = 32

_CACHE = {}


def _build_program():
    import concourse.bass as bass
    import concourse.tile as tile
    from concourse import bacc, mybir

    f32 = mybir.dt.float32
    f16 = mybir.dt.float16
    nc = bacc.Bacc("TRN2", target_bir_lowering=False, debug=False,
                   num_devices=NCORES)

    a_d = nc.dram_tensor("a", [FT, C, 512], f16, kind="ExternalInput").ap()
    x_d = nc.dram_tensor("x", [C, T], f16, kind="ExternalInput").ap()
    cphi_d = nc.dram_tensor("cphi", [FT, 128, 64], f32, kind="ExternalInput").ap()
    sphi_d = nc.dram_tensor("sphi", [FT, 128, 64], f32, kind="ExternalInput").ap()
    cpsphi_d = nc.dram_tensor("cpsphi", [FT, 128, 64], f32,
                              kind="ExternalInput").ap()
    mh_d = nc.dram_tensor("mh", [3, 128, 128], f16, kind="ExternalInput").ap()
    y_d = nc.dram_tensor("y", [BPC, P], f32, kind="ExternalOutput").ap()

    mult = mybir.AluOpType.mult
    add = mybir.AluOpType.add
    Act = mybir.ActivationFunctionType

    # plane m -> (psum pair tile, column offset); pairs are
    # T1 = [r1 | i1], T2 = [i2 | r2] (i2 first so the S_im pair-TTR reads
    # (r1,i1)x(i2,r2) with plain slices)
    with tile.TileContext(nc) as tc:
        with (
            tc.tile_pool(name="const", bufs=1) as const,
            tc.tile_pool(name="apool", bufs=3) as apool,
            tc.tile_pool(name="ps", bufs=3, space="PSUM") as pspool,
            tc.tile_pool(name="ps1", bufs=1, space="PSUM") as pspool1,
            tc.tile_pool(name="stg", bufs=2) as stg,
            tc.tile_pool(name="scr", bufs=3) as scr,
        ):
            # emission order = DMA queue order: first matmul needs x[ck0] + a0,
            # so those go first, bulky twiddle tables last
            x_src = x_d.rearrange("(ck p) t -> p ck t", p=128)
            x_sb = const.tile([128, 4, T], f16)
            nc.sync.dma_start(x_sb[:, 0], x_src[:, 0])
            a_tiles = {}
            for ft in range(3):
                a_t = apool.tile([128, 4, 512], f16, tag="a", name=f"a_{ft}")
                nc.sync.dma_start(
                    a_t[:], a_d[ft].rearrange("(ck p) m -> p ck m", p=128)
                )
                a_tiles[ft] = a_t
                if ft == 0:
                    nc.sync.dma_start(x_sb[:, 1:4], x_src[:, 1:4])
            cphi_sb = const.tile([128, FT, 64], f32)
            nc.sync.dma_start(cphi_sb[:], cphi_d.rearrange("kt p s -> p kt s"))
            sphi_sb = const.tile([128, FT, 64], f32)
            nc.sync.dma_start(sphi_sb[:], sphi_d.rearrange("kt p s -> p kt s"))
            cpsphi_sb = const.tile([128, FT, 64], f32)
            nc.sync.dma_start(cpsphi_sb[:], cpsphi_d.rearrange("kt p s -> p kt s"))
            mh_sb = const.tile([128, 3, 128], f16)
            nc.sync.dma_start(mh_sb[:], mh_d.rearrange("m p q -> p m q"))
            ones_sb = const.tile([128, 1], f32)
            nc.vector.memset(ones_sb[:], 1.0)
            # warm the ACT table set (sqrt anchor; abs/sign/copy ride along)
            warm = scr.tile([1, 1], f32, tag="warm")
            nc.vector.memset(warm[:], 1.0)
            nc.scalar.activation(warm[:], warm[:], Act.Sqrt)
            sre_sb = const.tile([128, FT * 4], f32)
            sim_sb = const.tile([128, FT * 4], f32)
            ssum_sb = const.tile([128, FT * 4], f32)
            accA_sb = const.tile([128, FT * 4], f32)
            accB_sb = const.tile([128, FT * 4], f32)
            tw_sb = const.tile([128, FT, 3, 4, 64], f16)

            # ---- stage A+B+twiddle, pipelined per frequency tile ----
            # per-plane PSUM tiles (2 banks each) rotate through 3 buffers;
            # psy owns its own bank so the stage-C IFFT matmuls interleave
            # with the loop (PSUM accumulation groups are bank-granular)
            psy = pspool1.tile([128, 512], f32, tag="psy")

            def stage_c(kt):
                for mat in range(3):
                    nc.tensor.matmul(
                        psy[:, 0:BPC * 64],
                        mh_sb[:, mat],
                        tw_sb[:, kt, mat].rearrange("p b s -> p (b s)"),
                        start=(mat == 0 and kt == 0),
                        stop=(mat == 2 and kt == FT - 1),
                    )

            for ft in range(FT):
                if ft in a_tiles:
                    a_t = a_tiles[ft]
                else:
                    a_t = apool.tile([128, 4, 512], f16, tag="a", name=f"a_{ft}")
                    nc.sync.dma_start(
                        a_t[:], a_d[ft].rearrange("(ck p) m -> p ck m", p=128)
                    )
                if ft >= 2:
                    stage_c(ft - 2)
                # m: 0->r1(t1sb:0) 1->i1(t1sb:1) 2->i2(t2sb:0) 3->r2(t2sb:1)
                # t2sb planes first so their drains overlap the t1 matmuls
                t2sb = stg.tile([128, 2, T], f16, tag="t2sb")
                t1sb = stg.tile([128, 2, T], f16, tag="t1sb")
                for m, sbt, half in ((2, t2sb, 0), (3, t2sb, 1),
                                     (0, t1sb, 0), (1, t1sb, 1)):
                    msl = slice(m * 128, (m + 1) * 128)
                    pt = pspool.tile([128, 1024], f32, tag="pl",
                                     name=f"pl_{ft}_{m}")
                    for ck in range(4):
                        for c0, cn in ((0, 512), (512, T - 512)):
                            nc.tensor.matmul(
                                pt[:, c0:c0 + cn],
                                a_t[:, ck, msl],
                                x_sb[:, ck, c0:c0 + cn],
                                start=(ck == 0),
                                stop=(ck == 3),
                            )
                    nc.scalar.activation(sbt[:, half], pt[:, 0:T], Act.Copy)
                # fused multiply+reduce (STT): S_re parts and S_im per batch elem
                for b in range(BPC):
                    bs = slice(b * HW, (b + 1) * HW)
                    slot = slice(ft * 4 + b, ft * 4 + b + 1)
                    po = scr.tile([128, HW], f16, tag=f"po{b}")
                    pp = scr.tile([128, 2, HW], f16, tag=f"pp{b}")
                    nc.vector.scalar_tensor_tensor(
                        out=po[:], in0=t1sb[:, 0, bs], scalar=1.0,
                        in1=t2sb[:, 1, bs], op0=mult, op1=mult,
                        accum_out=accA_sb[:, slot],
                    )
                    nc.vector.scalar_tensor_tensor(
                        out=po[:], in0=t1sb[:, 1, bs], scalar=1.0,
                        in1=t2sb[:, 0, bs], op0=mult, op1=mult,
                        accum_out=accB_sb[:, slot],
                    )
                    nc.vector.scalar_tensor_tensor(
                        out=pp[:], in0=t1sb[:, :, bs], scalar=1.0,
                        in1=t2sb[:, :, bs], op0=mult, op1=mult,
                        accum_out=sim_sb[:, slot],
                    )
                # Gauss twiddle products for this kt
                sl4 = slice(ft * 4, (ft + 1) * 4)
                nc.vector.tensor_sub(sre_sb[:, sl4], accA_sb[:, sl4],
                                     accB_sb[:, sl4])
                nc.vector.tensor_add(ssum_sb[:, sl4], sre_sb[:, sl4],
                                     sim_sb[:, sl4])
                for mat, tbl, s_in in ((0, cphi_sb, sre_sb),
                                       (1, sphi_sb, sim_sb),
                                       (2, cpsphi_sb, ssum_sb)):
                    nc.vector.tensor_tensor(
                        tw_sb[:, ft, mat],
                        tbl[:, ft, :][:, None, :].broadcast_to([128, BPC, 64]),
                        s_in[:, sl4][:, :, None].broadcast_to([128, BPC, 64]),
                        op=mult,
                    )

            # ---- stage C epilogue: last two frequency tiles ----
            stage_c(FT - 2)
            stage_c(FT - 1)

            # ---- stage D: signed sqrt, per-batch l2 norm, store ----
            absy = scr.tile([128, BPC * 64], f32, tag="absy")
            sgn = scr.tile([128, BPC * 64], f32, tag="sgn")
            nc.scalar.activation(absy[:], psy[:, 0:BPC * 64], Act.Abs)
            nc.scalar.activation(sgn[:], psy[:, 0:BPC * 64], Act.Sign)
            sqy = scr.tile([128, BPC * 64], f32, tag="sqy")
            nc.scalar.activation(sqy[:], absy[:], Act.Sqrt)
            ys = scr.tile([128, BPC * 64], f32, tag="ys")
            nc.vector.tensor_mul(ys[:], sqy[:], sgn[:])

            psnt = pspool1.tile([128, 512], f32, tag="psn")
            psn = psnt[0:1, 0:BPC * 64]
            nc.tensor.matmul(psn, ones_sb[:], absy[:],
                             start=True, stop=True)
            nsq = scr.tile([1, BPC], f32, tag="nsq")
            nc.vector.reduce_sum(
                out=nsq[:],
                in_=psn.rearrange("p (b s) -> p b s", b=BPC),
                axis=mybir.AxisListType.X,
            )
            nc.vector.tensor_scalar_max(nsq[:], nsq[:], 1e-10)
            sqn = scr.tile([1, BPC], f32, tag="sqn")
            nc.scalar.activation(sqn[:], nsq[:], Act.Sqrt)
            invn = scr.tile([1, BPC], f32, tag="invn")
            nc.vector.reciprocal(invn[:], sqn[:])

            onesrow = const.tile([1, 128], f32)
            nc.vector.memset(onesrow[:], 1.0)
            psb = psnt[:, 256:256 + BPC]
            nc.tensor.matmul(psb, onesrow[0:1, :], invn[0:1, :],
                             start=True, stop=True)
            inv_b = psb[:, :, None].broadcast_to([128, BPC, 64])
            fin = scr.tile([128, BPC * 64], f32, tag="fin")
            nc.vector.tensor_tensor(
                fin[:].rearrange("p (b s) -> p b s", b=BPC),
                ys[:].rearrange("p (b s) -> p b s", b=BPC),
                inv_b,
                op=mult,
            )
            nc.sync.dma_start(
                y_d.rearrange("b (q s) -> q b s", q=128),
                fin[:].rearrange("p (b s) -> p b s", b=BPC),
            )

    nc.compile()
    return nc


def _host_prep(x, M1, M2):
    x = np.ascontiguousarray(np.asarray(x, np.float32))
    M1 = np.asarray(M1, np.float32)
    M2 = np.asarray(M2, np.float32)

    h1 = np.argmax(np.abs(M1), axis=1)
    s1 = M1[np.arange(C), h1].astype(np.float64)
    h2 = np.argmax(np.abs(M2), axis=1)
    s2 = M2[np.arange(C), h2].astype(np.float64)

    k = np.arange(NSLOT, dtype=np.float64)
    valid = k <= P // 2
    ang1 = 2 * np.pi * np.outer(h1.astype(np.float64), k) / P
    ang2 = 2 * np.pi * np.outer(h2.astype(np.float64), k) / P
    # a[ft, c, m*128 + j]: m planes (A1re, A1im, A2im, A2re), freq = ft*128 + j
    a = np.empty((FT, C, 512), np.float16)
    a1re = (s1[:, None] * np.cos(ang1) * valid).astype(np.float16)
    a1im = (-s1[:, None] * np.sin(ang1) * valid).astype(np.float16)
    a2re = (s2[:, None] * np.cos(ang2) * valid).astype(np.float16)
    a2im = (-s2[:, None] * np.sin(ang2) * valid).astype(np.float16)
    for ft in range(FT):
        ksl = slice(ft * 128, (ft + 1) * 128)
        a[ft, :, 0:128] = a1re[:, ksl]
        a[ft, :, 128:256] = a1im[:, ksl]
        a[ft, :, 256:384] = a2im[:, ksl]
        a[ft, :, 384:512] = a2re[:, ksl]

    w = np.where(valid, 2.0 / P, 0.0)
    w[0] = 1.0 / P
    w[P // 2] = 1.0 / P
    s_idx = np.arange(64, dtype=np.float64)
    phi = 2 * np.pi * np.outer(k, s_idx) / P
    cphi = (w[:, None] * np.cos(phi)).astype(np.float32).reshape(FT, 128, 64)
    sphi = (w[:, None] * np.sin(phi)).astype(np.float32).reshape(FT, 128, 64)
    cpsphi = (w[:, None] * (np.cos(phi) + np.sin(phi))).astype(
        np.float32).reshape(FT, 128, 64)

    km = np.arange(128, dtype=np.float64)
    alpha = 2 * np.pi * np.outer(km, km) / 128
    mh = np.empty((3, 128, 128), np.float16)
    mh[0] = (np.cos(alpha) + np.sin(alpha)).astype(np.float16)   # cosa - nsina
    mh[1] = (np.sin(alpha) - np.cos(alpha)).astype(np.float16)   # -(cosa+nsina)
    mh[2] = (-np.sin(alpha)).astype(np.float16)                  # nsina

    xt = np.ascontiguousarray(x.reshape(B * HW, C).T).astype(np.float16)

    return a, cphi, sphi, cpsphi, mh, xt


def _make_in_maps(x, M1, M2):
    a, cphi, sphi, cpsphi, mh, xt = _host_prep(x, M1, M2)
    in_maps = []
    for r in range(NCORES):
        in_maps.append({
            "a": a,
            "x": np.ascontiguousarray(xt[:, r * T:(r + 1) * T]),
            "cphi": cphi,
            "sphi": sphi,
            "cpsphi": cpsphi,
            "mh": mh,
        })
    return in_maps


def kernel(x, M1, M2):
    from concourse.bass_utils import run_bass_kernel_spmd

    if "nc" not in _CACHE:
        _CACHE["nc"] = _build_program()
    nc = _CACHE["nc"]

    in_maps = _make_in_maps(x, M1, M2)
    res = run_bass_kernel_spmd(nc, in_maps, core_ids=list(range(NCORES)))
    out = np.concatenate([res.results[r]["y"] for r in range(NCORES)], axis=0)
    return out.astype(np.float32)


# revision 20
# speedup vs baseline: 1.0093x; 1.0093x over previous
"""Trainium2 kernel for CompactBilinearLayer (count-sketch bilinear pooling).

Math: reference computes y = l2norm(signed_sqrt(sum_hw Re IFFT(FFT(x@M1)*FFT(x@M2)))).
Since M1/M2 are count-sketch matrices (one +-1 per row), FFT(x@M1) == x @ A1 with
A1[c,k] = s1[c] * exp(-2pi i h1[c] k / P) — a dense [512, K] matrix computable on the
host from M1 in O(C*K). The IFFT is linear, so the spatial sum moves before it.
Hermitian symmetry means only k = 0..4096 are needed.  Per core (4 batch elements,
784 spatial positions — fully batch-local, no collectives):
  A: projections r1,i1,i2,r2 = x @ A-planes, single fp16 matmul each (rel-err
     budget 2e-2 allows it; measured ~1.2e-3 end to end)
  B: S[k,b] = sum_t P1*P2: ScalarE drains PSUM planes to SBUF fp16, then DVE
     tensor_tensor_reduce chains (fused multiply+reduce, fp32 accum) produce
     S_re = sum r1r2 - i1i2 and S_im = sum r1i2 + i1r2 directly
  C: IFFT via two-step factorization p=64q+s: Gauss 3-mult twiddle
     t1=cphi*S_re, t2=sphi*S_im, t3=(cphi+sphi)*(S_re+S_im), then
     psy = (cosa-nsina)@t1 - (cosa+nsina)@t2 + nsina@t3  (128-DFT matmuls)
  D: signed sqrt + per-batch L2 norm + store
"""
import numpy as np

P = 8192
C = 512
FT = 33            # frequency tiles of 128 -> 4224 slots >= 4097
NSLOT = FT * 128
NCORES = 8
BPC = 4            # batch elems per core
HW = 196           # spatial positions per batch elem
T = BPC * HW       # 784 positions per core
B = 32

_CACHE = {}


def _build_program():
    import concourse.bass as bass
    import concourse.tile as tile
    from concourse import bacc, mybir

    f32 = mybir.dt.float32
    f16 = mybir.dt.float16
    nc = bacc.Bacc("TRN2", target_bir_lowering=False, debug=False,
                   num_devices=NCORES)

    a_d = nc.dram_tensor("a", [FT, C, 512], f16, kind="ExternalInput").ap()
    x_d = nc.dram_tensor("x", [C, T], f16, kind="ExternalInput").ap()
    cphi_d = nc.dram_tensor("cphi", [FT, 128, 64], f32, kind="ExternalInput").ap()
    sphi_d = nc.dram_tensor("sphi", [FT, 128, 64], f32, kind="ExternalInput").ap()
    cpsphi_d = nc.dram_tensor("cpsphi", [FT, 128, 64], f32,
                              kind="ExternalInput").ap()
    mh_d = nc.dram_tensor("mh", [3, 128, 128], f16, kind="ExternalInput").ap()
    y_d = nc.dram_tensor("y", [BPC, P], f32, kind="ExternalOutput").ap()

    mult = mybir.AluOpType.mult
    add = mybir.AluOpType.add
    Act = mybir.ActivationFunctionType

    # plane m -> (psum pair tile, column offset); pairs are
    # T1 = [r1 | i1], T2 = [i2 | r2] (i2 first so the S_im pair-TTR reads
    # (r1,i1)x(i2,r2) with plain slices)
    with tile.TileContext(nc) as tc:
        with (
            tc.tile_pool(name="const", bufs=1) as const,
            tc.tile_pool(name="apool", bufs=3) as apool,
            tc.tile_pool(name="ps", bufs=3, space="PSUM") as pspool,
            tc.tile_pool(name="ps1", bufs=1, space="PSUM") as pspool1,
            tc.tile_pool(name="stg", bufs=2) as stg,
            tc.tile_pool(name="scr", bufs=3) as scr,
        ):
            # emission order = DMA queue order: first matmul needs x[ck0] + a0,
            # so those go first, bulky twiddle tables last
            x_src = x_d.rearrange("(ck p) t -> p ck t", p=128)
            x_sb = const.tile([128, 4, T], f16)
            nc.sync.dma_start(x_sb[:, 0], x_src[:, 0])
            a_tiles = {}
            for ft in range(3):
                a_t = apool.tile([128, 4, 512], f16, tag="a", name=f"a_{ft}")
                nc.sync.dma_start(
                    a_t[:], a_d[ft].rearrange("(ck p) m -> p ck m", p=128)
                )
                a_tiles[ft] = a_t
                if ft == 0:
                    nc.sync.dma_start(x_sb[:, 1:4], x_src[:, 1:4])
            cphi_sb = const.tile([128, FT, 64], f32)
            nc.sync.dma_start(cphi_sb[:], cphi_d.rearrange("kt p s -> p kt s"))
            sphi_sb = const.tile([128, FT, 64], f32)
            nc.sync.dma_start(sphi_sb[:], sphi_d.rearrange("kt p s -> p kt s"))
            cpsphi_sb = const.tile([128, FT, 64], f32)
            nc.sync.dma_start(cpsphi_sb[:], cpsphi_d.rearrange("kt p s -> p kt s"))
            mh_sb = const.tile([128, 3, 128], f16)
            nc.sync.dma_start(mh_sb[:], mh_d.rearrange("m p q -> p m q"))
            ones_sb = const.tile([128, 1], f32)
            nc.vector.memset(ones_sb[:], 1.0)
            # warm the ACT table set (sqrt anchor; abs/sign/copy ride along)
            warm = scr.tile([1, 1], f32, tag="warm")
            nc.vector.memset(warm[:], 1.0)
            nc.scalar.activation(warm[:], warm[:], Act.Sqrt)
            sre_sb = const.tile([128, FT * 4], f32)
            sim_sb = const.tile([128, FT * 4], f32)
            ssum_sb = const.tile([128, FT * 4], f32)
            accA_sb = const.tile([128, FT * 4], f32)
            accB_sb = const.tile([128, FT * 4], f32)
            tw_sb = const.tile([128, FT, 3, 4, 64], f16)

            # ---- stage A+B+twiddle, pipelined per frequency tile ----
            # per-plane PSUM tiles (2 banks each) rotate through 3 buffers;
            # psy owns its own bank so the stage-C IFFT matmuls interleave
            # with the loop (PSUM accumulation groups are bank-granular)
            psy = pspool1.tile([128, 512], f32, tag="psy")

            def stage_c(kt):
                for mat in range(3):
                    nc.tensor.matmul(
                        psy[:, 0:BPC * 64],
                        mh_sb[:, mat],
                        tw_sb[:, kt, mat].rearrange("p b s -> p (b s)"),
                        start=(mat == 0 and kt == 0),
                        stop=(mat == 2 and kt == FT - 1),
                    )

            for ft in range(FT):
                if ft in a_tiles:
                    a_t = a_tiles[ft]
                else:
                    a_t = apool.tile([128, 4, 512], f16, tag="a", name=f"a_{ft}")
                    nc.sync.dma_start(
                        a_t[:], a_d[ft].rearrange("(ck p) m -> p ck m", p=128)
                    )
                if ft >= 4:
                    stage_c(ft - 4)
                # m: 0->r1(t1sb:0) 1->i1(t1sb:1) 2->i2(t2sb:0) 3->r2(t2sb:1)
                # t2sb planes first so their drains overlap the t1 matmuls
                t2sb = stg.tile([128, 2, T], f16, tag="t2sb")
                t1sb = stg.tile([128, 2, T], f16, tag="t1sb")
                for m, sbt, half in ((2, t2sb, 0), (3, t2sb, 1),
                                     (0, t1sb, 0), (1, t1sb, 1)):
                    msl = slice(m * 128, (m + 1) * 128)
                    pt = pspool.tile([128, 1024], f32, tag="pl",
                                     name=f"pl_{ft}_{m}")
                    for ck in range(4):
                        for c0, cn in ((0, 512), (512, T - 512)):
                            nc.tensor.matmul(
                                pt[:, c0:c0 + cn],
                                a_t[:, ck, msl],
                                x_sb[:, ck, c0:c0 + cn],
                                start=(ck == 0),
                                stop=(ck == 3),
                            )
                    nc.scalar.activation(sbt[:, half], pt[:, 0:T], Act.Copy)
                # fused multiply+reduce (STT): S_re parts and S_im per batch elem
                for b in range(BPC):
                    bs = slice(b * HW, (b + 1) * HW)
                    slot = slice(ft * 4 + b, ft * 4 + b + 1)
                    po = scr.tile([128, HW], f16, tag=f"po{b}")
                    pp = scr.tile([128, 2, HW], f16, tag=f"pp{b}")
                    nc.vector.scalar_tensor_tensor(
                        out=po[:], in0=t1sb[:, 0, bs], scalar=1.0,
                        in1=t2sb[:, 1, bs], op0=mult, op1=mult,
                        accum_out=accA_sb[:, slot],
                    )
                    nc.vector.scalar_tensor_tensor(
                        out=po[:], in0=t1sb[:, 1, bs], scalar=1.0,
                        in1=t2sb[:, 0, bs], op0=mult, op1=mult,
                        accum_out=accB_sb[:, slot],
                    )
                    nc.vector.scalar_tensor_tensor(
                        out=pp[:], in0=t1sb[:, :, bs], scalar=1.0,
                        in1=t2sb[:, :, bs], op0=mult, op1=mult,
                        accum_out=sim_sb[:, slot],
                    )
                # Gauss twiddle products for this kt
                sl4 = slice(ft * 4, (ft + 1) * 4)
                nc.vector.tensor_sub(sre_sb[:, sl4], accA_sb[:, sl4],
                                     accB_sb[:, sl4])
                nc.vector.tensor_add(ssum_sb[:, sl4], sre_sb[:, sl4],
                                     sim_sb[:, sl4])
                for mat, tbl, s_in in ((0, cphi_sb, sre_sb),
                                       (1, sphi_sb, sim_sb),
                                       (2, cpsphi_sb, ssum_sb)):
                    nc.vector.tensor_tensor(
                        tw_sb[:, ft, mat],
                        tbl[:, ft, :][:, None, :].broadcast_to([128, BPC, 64]),
                        s_in[:, sl4][:, :, None].broadcast_to([128, BPC, 64]),
                        op=mult,
                    )

            # ---- stage C epilogue: remaining frequency tiles ----
            for kt in range(FT - 4, FT):
                stage_c(kt)

            # ---- stage D: signed sqrt, per-batch l2 norm, store ----
            absy = scr.tile([128, BPC * 64], f32, tag="absy")
            sgn = scr.tile([128, BPC * 64], f32, tag="sgn")
            nc.scalar.activation(absy[:], psy[:, 0:BPC * 64], Act.Abs)
            nc.scalar.activation(sgn[:], psy[:, 0:BPC * 64], Act.Sign)
            sqy = scr.tile([128, BPC * 64], f32, tag="sqy")
            nc.scalar.activation(sqy[:], absy[:], Act.Sqrt)
            ys = scr.tile([128, BPC * 64], f32, tag="ys")
            nc.vector.tensor_mul(ys[:], sqy[:], sgn[:])

            psnt = pspool1.tile([128, 512], f32, tag="psn")
            psn = psnt[0:1, 0:BPC * 64]
            nc.tensor.matmul(psn, ones_sb[:], absy[:],
                             start=True, stop=True)
            nsq = scr.tile([1, BPC], f32, tag="nsq")
            nc.vector.reduce_sum(
                out=nsq[:],
                in_=psn.rearrange("p (b s) -> p b s", b=BPC),
                axis=mybir.AxisListType.X,
            )
            nc.vector.tensor_scalar_max(nsq[:], nsq[:], 1e-10)
            sqn = scr.tile([1, BPC], f32, tag="sqn")
            nc.scalar.activation(sqn[:], nsq[:], Act.Sqrt)
            invn = scr.tile([1, BPC], f32, tag="invn")
            nc.vector.reciprocal(invn[:], sqn[:])

            onesrow = const.tile([1, 128], f32)
            nc.vector.memset(onesrow[:], 1.0)
            psb = psnt[:, 256:256 + BPC]
            nc.tensor.matmul(psb, onesrow[0:1, :], invn[0:1, :],
                             start=True, stop=True)
            inv_b = psb[:, :, None].broadcast_to([128, BPC, 64])
            fin = scr.tile([128, BPC * 64], f32, tag="fin")
            nc.vector.tensor_tensor(
                fin[:].rearrange("p (b s) -> p b s", b=BPC),
                ys[:].rearrange("p (b s) -> p b s", b=BPC),
                inv_b,
                op=mult,
            )
            nc.sync.dma_start(
                y_d.rearrange("b (q s) -> q b s", q=128),
                fin[:].rearrange("p (b s) -> p b s", b=BPC),
            )

    nc.compile()
    return nc


def _host_prep(x, M1, M2):
    x = np.ascontiguousarray(np.asarray(x, np.float32))
    M1 = np.asarray(M1, np.float32)
    M2 = np.asarray(M2, np.float32)

    h1 = np.argmax(np.abs(M1), axis=1)
    s1 = M1[np.arange(C), h1].astype(np.float64)
    h2 = np.argmax(np.abs(M2), axis=1)
    s2 = M2[np.arange(C), h2].astype(np.float64)

    k = np.arange(NSLOT, dtype=np.float64)
    valid = k <= P // 2
    ang1 = 2 * np.pi * np.outer(h1.astype(np.float64), k) / P
    ang2 = 2 * np.pi * np.outer(h2.astype(np.float64), k) / P
    # a[ft, c, m*128 + j]: m planes (A1re, A1im, A2im, A2re), freq = ft*128 + j
    a = np.empty((FT, C, 512), np.float16)
    a1re = (s1[:, None] * np.cos(ang1) * valid).astype(np.float16)
    a1im = (-s1[:, None] * np.sin(ang1) * valid).astype(np.float16)
    a2re = (s2[:, None] * np.cos(ang2) * valid).astype(np.float16)
    a2im = (-s2[:, None] * np.sin(ang2) * valid).astype(np.float16)
    for ft in range(FT):
        ksl = slice(ft * 128, (ft + 1) * 128)
        a[ft, :, 0:128] = a1re[:, ksl]
        a[ft, :, 128:256] = a1im[:, ksl]
        a[ft, :, 256:384] = a2im[:, ksl]
        a[ft, :, 384:512] = a2re[:, ksl]

    w = np.where(valid, 2.0 / P, 0.0)
    w[0] = 1.0 / P
    w[P // 2] = 1.0 / P
    s_idx = np.arange(64, dtype=np.float64)
    phi = 2 * np.pi * np.outer(k, s_idx) / P
    cphi = (w[:, None] * np.cos(phi)).astype(np.float32).reshape(FT, 128, 64)
    sphi = (w[:, None] * np.sin(phi)).astype(np.float32).reshape(FT, 128, 64)
    cpsphi = (w[:, None] * (np.cos(phi) + np.sin(phi))).astype(
        np.float32).reshape(FT, 128, 64)

    km = np.arange(128, dtype=np.float64)
    alpha = 2 * np.pi * np.outer(km, km) / 128
    mh = np.empty((3, 128, 128), np.float16)
    mh[0] = (np.cos(alpha) + np.sin(alpha)).astype(np.float16)   # cosa - nsina
    mh[1] = (np.sin(alpha) - np.cos(alpha)).astype(np.float16)   # -(cosa+nsina)
    mh[2] = (-np.sin(alpha)).astype(np.float16)                  # nsina

    xt = np.ascontiguousarray(x.reshape(B * HW, C).T).astype(np.float16)

    return a, cphi, sphi, cpsphi, mh, xt


def _make_in_maps(x, M1, M2):
    a, cphi, sphi, cpsphi, mh, xt = _host_prep(x, M1, M2)
    in_maps = []
    for r in range(NCORES):
        in_maps.append({
            "a": a,
            "x": np.ascontiguousarray(xt[:, r * T:(r + 1) * T]),
            "cphi": cphi,
            "sphi": sphi,
            "cpsphi": cpsphi,
            "mh": mh,
        })
    return in_maps


def kernel(x, M1, M2):
    from concourse.bass_utils import run_bass_kernel_spmd

    if "nc" not in _CACHE:
        _CACHE["nc"] = _build_program()
    nc = _CACHE["nc"]

    in_maps = _make_in_maps(x, M1, M2)
    res = run_bass_kernel_spmd(nc, in_maps, core_ids=list(range(NCORES)))
    out = np.concatenate([res.results[r]["y"] for r in range(NCORES)], axis=0)
    return out.astype(np.float32)


# revision 21
# speedup vs baseline: 1.0125x; 1.0031x over previous
"""Trainium2 kernel for CompactBilinearLayer (count-sketch bilinear pooling).

Math: reference computes y = l2norm(signed_sqrt(sum_hw Re IFFT(FFT(x@M1)*FFT(x@M2)))).
Since M1/M2 are count-sketch matrices (one +-1 per row), FFT(x@M1) == x @ A1 with
A1[c,k] = s1[c] * exp(-2pi i h1[c] k / P) — a dense [512, K] matrix computable on the
host from M1 in O(C*K). The IFFT is linear, so the spatial sum moves before it.
Hermitian symmetry means only k = 0..4096 are needed.  Per core (4 batch elements,
784 spatial positions — fully batch-local, no collectives):
  A: projections r1,i1,i2,r2 = x @ A-planes, single fp16 matmul each (rel-err
     budget 2e-2 allows it; measured ~1.2e-3 end to end)
  B: S[k,b] = sum_t P1*P2: ScalarE drains PSUM planes to SBUF fp16, then DVE
     tensor_tensor_reduce chains (fused multiply+reduce, fp32 accum) produce
     S_re = sum r1r2 - i1i2 and S_im = sum r1i2 + i1r2 directly
  C: IFFT via two-step factorization p=64q+s: Gauss 3-mult twiddle
     t1=cphi*S_re, t2=sphi*S_im, t3=(cphi+sphi)*(S_re+S_im), then
     psy = (cosa-nsina)@t1 - (cosa+nsina)@t2 + nsina@t3  (128-DFT matmuls)
  D: signed sqrt + per-batch L2 norm + store
"""
import numpy as np

P = 8192
C = 512
FT = 33            # frequency tiles of 128 -> 4224 slots >= 4097
NSLOT = FT * 128
NCORES = 8
BPC = 4            # batch elems per core
HW = 196           # spatial positions per batch elem
T = BPC * HW       # 784 positions per core
B = 32

_CACHE = {}


def _build_program():
    import concourse.bass as bass
    import concourse.tile as tile
    from concourse import bacc, mybir

    f32 = mybir.dt.float32
    f16 = mybir.dt.float16
    nc = bacc.Bacc("TRN2", target_bir_lowering=False, debug=False,
                   num_devices=NCORES)

    a_d = nc.dram_tensor("a", [FT, C, 512], f16, kind="ExternalInput").ap()
    x_d = nc.dram_tensor("x", [C, T], f16, kind="ExternalInput").ap()
    cphi_d = nc.dram_tensor("cphi", [FT, 128, 64], f32, kind="ExternalInput").ap()
    sphi_d = nc.dram_tensor("sphi", [FT, 128, 64], f32, kind="ExternalInput").ap()
    cpsphi_d = nc.dram_tensor("cpsphi", [FT, 128, 64], f32,
                              kind="ExternalInput").ap()
    mh_d = nc.dram_tensor("mh", [3, 128, 128], f16, kind="ExternalInput").ap()
    y_d = nc.dram_tensor("y", [BPC, P], f32, kind="ExternalOutput").ap()

    mult = mybir.AluOpType.mult
    add = mybir.AluOpType.add
    Act = mybir.ActivationFunctionType

    # plane m -> (psum pair tile, column offset); pairs are
    # T1 = [r1 | i1], T2 = [i2 | r2] (i2 first so the S_im pair-TTR reads
    # (r1,i1)x(i2,r2) with plain slices)
    with tile.TileContext(nc) as tc:
        with (
            tc.tile_pool(name="const", bufs=1) as const,
            tc.tile_pool(name="apool", bufs=3) as apool,
            tc.tile_pool(name="ps", bufs=3, space="PSUM") as pspool,
            tc.tile_pool(name="ps1", bufs=1, space="PSUM") as pspool1,
            tc.tile_pool(name="stg", bufs=2) as stg,
            tc.tile_pool(name="scr", bufs=3) as scr,
        ):
            # emission order = DMA queue order: first matmul needs x[ck0] + a0,
            # so those go first, bulky twiddle tables last
            x_src = x_d.rearrange("(ck p) t -> p ck t", p=128)
            x_sb = const.tile([128, 4, T], f16)
            nc.sync.dma_start(x_sb[:, 0], x_src[:, 0])
            a_tiles = {}
            for ft in range(3):
                a_t = apool.tile([128, 4, 512], f16, tag="a", name=f"a_{ft}")
                nc.sync.dma_start(
                    a_t[:], a_d[ft].rearrange("(ck p) m -> p ck m", p=128)
                )
                a_tiles[ft] = a_t
                if ft == 0:
                    nc.sync.dma_start(x_sb[:, 1:4], x_src[:, 1:4])
            cphi_sb = const.tile([128, FT, 64], f32)
            nc.sync.dma_start(cphi_sb[:], cphi_d.rearrange("kt p s -> p kt s"))
            sphi_sb = const.tile([128, FT, 64], f32)
            nc.sync.dma_start(sphi_sb[:], sphi_d.rearrange("kt p s -> p kt s"))
            cpsphi_sb = const.tile([128, FT, 64], f32)
            nc.sync.dma_start(cpsphi_sb[:], cpsphi_d.rearrange("kt p s -> p kt s"))
            mh_sb = const.tile([128, 3, 128], f16)
            nc.sync.dma_start(mh_sb[:], mh_d.rearrange("m p q -> p m q"))
            ones_sb = const.tile([128, 1], f32)
            nc.vector.memset(ones_sb[:], 1.0)
            # warm the ACT table set (sqrt anchor; abs/sign/copy ride along)
            warm = scr.tile([1, 1], f32, tag="warm")
            nc.vector.memset(warm[:], 1.0)
            nc.scalar.activation(warm[:], warm[:], Act.Sqrt)
            sre_sb = const.tile([128, FT * 4], f32)
            sim_sb = const.tile([128, FT * 4], f32)
            ssum_sb = const.tile([128, FT * 4], f32)
            accA_sb = const.tile([128, FT * 4], f32)
            accB_sb = const.tile([128, FT * 4], f32)
            tw_sb = const.tile([128, FT, 3, 4, 64], f16)

            # ---- stage A+B+twiddle, pipelined per frequency tile ----
            # per-plane PSUM tiles (2 banks each) rotate through 3 buffers;
            # psy owns its own bank so the stage-C IFFT matmuls interleave
            # with the loop (PSUM accumulation groups are bank-granular)
            psy = pspool1.tile([128, 512], f32, tag="psy")

            def stage_c(kt):
                for mat in range(3):
                    nc.tensor.matmul(
                        psy[:, 0:BPC * 64],
                        mh_sb[:, mat],
                        tw_sb[:, kt, mat].rearrange("p b s -> p (b s)"),
                        start=(mat == 0 and kt == 0),
                        stop=(mat == 2 and kt == FT - 1),
                    )

            for ft in range(FT):
                if ft in a_tiles:
                    a_t = a_tiles[ft]
                else:
                    a_t = apool.tile([128, 4, 512], f16, tag="a", name=f"a_{ft}")
                    nc.sync.dma_start(
                        a_t[:], a_d[ft].rearrange("(ck p) m -> p ck m", p=128)
                    )
                if ft >= 8:
                    stage_c(ft - 8)
                # m: 0->r1(t1sb:0) 1->i1(t1sb:1) 2->i2(t2sb:0) 3->r2(t2sb:1)
                # t2sb planes first so their drains overlap the t1 matmuls
                t2sb = stg.tile([128, 2, T], f16, tag="t2sb")
                t1sb = stg.tile([128, 2, T], f16, tag="t1sb")
                for m, sbt, half in ((2, t2sb, 0), (3, t2sb, 1),
                                     (0, t1sb, 0), (1, t1sb, 1)):
                    msl = slice(m * 128, (m + 1) * 128)
                    pt = pspool.tile([128, 1024], f32, tag="pl",
                                     name=f"pl_{ft}_{m}")
                    for ck in range(4):
                        for c0, cn in ((0, 512), (512, T - 512)):
                            nc.tensor.matmul(
                                pt[:, c0:c0 + cn],
                                a_t[:, ck, msl],
                                x_sb[:, ck, c0:c0 + cn],
                                start=(ck == 0),
                                stop=(ck == 3),
                            )
                    nc.scalar.activation(sbt[:, half], pt[:, 0:T], Act.Copy)
                # fused multiply+reduce (STT): S_re parts and S_im per batch elem
                for b in range(BPC):
                    bs = slice(b * HW, (b + 1) * HW)
                    slot = slice(ft * 4 + b, ft * 4 + b + 1)
                    po = scr.tile([128, HW], f16, tag=f"po{b}")
                    pp = scr.tile([128, 2, HW], f16, tag=f"pp{b}")
                    nc.vector.scalar_tensor_tensor(
                        out=po[:], in0=t1sb[:, 0, bs], scalar=1.0,
                        in1=t2sb[:, 1, bs], op0=mult, op1=mult,
                        accum_out=accA_sb[:, slot],
                    )
                    nc.vector.scalar_tensor_tensor(
                        out=po[:], in0=t1sb[:, 1, bs], scalar=1.0,
                        in1=t2sb[:, 0, bs], op0=mult, op1=mult,
                        accum_out=accB_sb[:, slot],
                    )
                    nc.vector.scalar_tensor_tensor(
                        out=pp[:], in0=t1sb[:, :, bs], scalar=1.0,
                        in1=t2sb[:, :, bs], op0=mult, op1=mult,
                        accum_out=sim_sb[:, slot],
                    )
                # Gauss twiddle products for this kt
                sl4 = slice(ft * 4, (ft + 1) * 4)
                nc.vector.tensor_sub(sre_sb[:, sl4], accA_sb[:, sl4],
                                     accB_sb[:, sl4])
                nc.vector.tensor_add(ssum_sb[:, sl4], sre_sb[:, sl4],
                                     sim_sb[:, sl4])
                for mat, tbl, s_in in ((0, cphi_sb, sre_sb),
                                       (1, sphi_sb, sim_sb),
                                       (2, cpsphi_sb, ssum_sb)):
                    nc.vector.tensor_tensor(
                        tw_sb[:, ft, mat],
                        tbl[:, ft, :][:, None, :].broadcast_to([128, BPC, 64]),
                        s_in[:, sl4][:, :, None].broadcast_to([128, BPC, 64]),
                        op=mult,
                    )

            # ---- stage C epilogue: remaining frequency tiles ----
            for kt in range(FT - 8, FT):
                stage_c(kt)

            # ---- stage D: signed sqrt, per-batch l2 norm, store ----
            absy = scr.tile([128, BPC * 64], f32, tag="absy")
            sgn = scr.tile([128, BPC * 64], f32, tag="sgn")
            nc.scalar.activation(absy[:], psy[:, 0:BPC * 64], Act.Abs)
            nc.scalar.activation(sgn[:], psy[:, 0:BPC * 64], Act.Sign)
            sqy = scr.tile([128, BPC * 64], f32, tag="sqy")
            nc.scalar.activation(sqy[:], absy[:], Act.Sqrt)
            ys = scr.tile([128, BPC * 64], f32, tag="ys")
            nc.vector.tensor_mul(ys[:], sqy[:], sgn[:])

            psnt = pspool1.tile([128, 512], f32, tag="psn")
            psn = psnt[0:1, 0:BPC * 64]
            nc.tensor.matmul(psn, ones_sb[:], absy[:],
                             start=True, stop=True)
            nsq = scr.tile([1, BPC], f32, tag="nsq")
            nc.vector.reduce_sum(
                out=nsq[:],
                in_=psn.rearrange("p (b s) -> p b s", b=BPC),
                axis=mybir.AxisListType.X,
            )
            nc.vector.tensor_scalar_max(nsq[:], nsq[:], 1e-10)
            sqn = scr.tile([1, BPC], f32, tag="sqn")
            nc.scalar.activation(sqn[:], nsq[:], Act.Sqrt)
            invn = scr.tile([1, BPC], f32, tag="invn")
            nc.vector.reciprocal(invn[:], sqn[:])

            onesrow = const.tile([1, 128], f32)
            nc.vector.memset(onesrow[:], 1.0)
            psb = psnt[:, 256:256 + BPC]
            nc.tensor.matmul(psb, onesrow[0:1, :], invn[0:1, :],
                             start=True, stop=True)
            inv_b = psb[:, :, None].broadcast_to([128, BPC, 64])
            fin = scr.tile([128, BPC * 64], f32, tag="fin")
            nc.vector.tensor_tensor(
                fin[:].rearrange("p (b s) -> p b s", b=BPC),
                ys[:].rearrange("p (b s) -> p b s", b=BPC),
                inv_b,
                op=mult,
            )
            nc.sync.dma_start(
                y_d.rearrange("b (q s) -> q b s", q=128),
                fin[:].rearrange("p (b s) -> p b s", b=BPC),
            )

    nc.compile()
    return nc


def _host_prep(x, M1, M2):
    x = np.ascontiguousarray(np.asarray(x, np.float32))
    M1 = np.asarray(M1, np.float32)
    M2 = np.asarray(M2, np.float32)

    h1 = np.argmax(np.abs(M1), axis=1)
    s1 = M1[np.arange(C), h1].astype(np.float64)
    h2 = np.argmax(np.abs(M2), axis=1)
    s2 = M2[np.arange(C), h2].astype(np.float64)

    k = np.arange(NSLOT, dtype=np.float64)
    valid = k <= P // 2
    ang1 = 2 * np.pi * np.outer(h1.astype(np.float64), k) / P
    ang2 = 2 * np.pi * np.outer(h2.astype(np.float64), k) / P
    # a[ft, c, m*128 + j]: m planes (A1re, A1im, A2im, A2re), freq = ft*128 + j
    a = np.empty((FT, C, 512), np.float16)
    a1re = (s1[:, None] * np.cos(ang1) * valid).astype(np.float16)
    a1im = (-s1[:, None] * np.sin(ang1) * valid).astype(np.float16)
    a2re = (s2[:, None] * np.cos(ang2) * valid).astype(np.float16)
    a2im = (-s2[:, None] * np.sin(ang2) * valid).astype(np.float16)
    for ft in range(FT):
        ksl = slice(ft * 128, (ft + 1) * 128)
        a[ft, :, 0:128] = a1re[:, ksl]
        a[ft, :, 128:256] = a1im[:, ksl]
        a[ft, :, 256:384] = a2im[:, ksl]
        a[ft, :, 384:512] = a2re[:, ksl]

    w = np.where(valid, 2.0 / P, 0.0)
    w[0] = 1.0 / P
    w[P // 2] = 1.0 / P
    s_idx = np.arange(64, dtype=np.float64)
    phi = 2 * np.pi * np.outer(k, s_idx) / P
    cphi = (w[:, None] * np.cos(phi)).astype(np.float32).reshape(FT, 128, 64)
    sphi = (w[:, None] * np.sin(phi)).astype(np.float32).reshape(FT, 128, 64)
    cpsphi = (w[:, None] * (np.cos(phi) + np.sin(phi))).astype(
        np.float32).reshape(FT, 128, 64)

    km = np.arange(128, dtype=np.float64)
    alpha = 2 * np.pi * np.outer(km, km) / 128
    mh = np.empty((3, 128, 128), np.float16)
    mh[0] = (np.cos(alpha) + np.sin(alpha)).astype(np.float16)   # cosa - nsina
    mh[1] = (np.sin(alpha) - np.cos(alpha)).astype(np.float16)   # -(cosa+nsina)
    mh[2] = (-np.sin(alpha)).astype(np.float16)                  # nsina

    xt = np.ascontiguousarray(x.reshape(B * HW, C).T).astype(np.float16)

    return a, cphi, sphi, cpsphi, mh, xt


def _make_in_maps(x, M1, M2):
    a, cphi, sphi, cpsphi, mh, xt = _host_prep(x, M1, M2)
    in_maps = []
    for r in range(NCORES):
        in_maps.append({
            "a": a,
            "x": np.ascontiguousarray(xt[:, r * T:(r + 1) * T]),
            "cphi": cphi,
            "sphi": sphi,
            "cpsphi": cpsphi,
            "mh": mh,
        })
    return in_maps


def kernel(x, M1, M2):
    from concourse.bass_utils import run_bass_kernel_spmd

    if "nc" not in _CACHE:
        _CACHE["nc"] = _build_program()
    nc = _CACHE["nc"]

    in_maps = _make_in_maps(x, M1, M2)
    res = run_bass_kernel_spmd(nc, in_maps, core_ids=list(range(NCORES)))
    out = np.concatenate([res.results[r]["y"] for r in range(NCORES)], axis=0)
    return out.astype(np.float32)


# revision 22
# speedup vs baseline: 1.0330x; 1.0203x over previous
"""Trainium2 kernel for CompactBilinearLayer (count-sketch bilinear pooling).

Math: reference computes y = l2norm(signed_sqrt(sum_hw Re IFFT(FFT(x@M1)*FFT(x@M2)))).
Since M1/M2 are count-sketch matrices (one +-1 per row), FFT(x@M1) == x @ A1 with
A1[c,k] = s1[c] * exp(-2pi i h1[c] k / P) — a dense [512, K] matrix computable on the
host from M1 in O(C*K). The IFFT is linear, so the spatial sum moves before it.
Hermitian symmetry means only k = 0..4096 are needed.  Per core (4 batch elements,
784 spatial positions — fully batch-local, no collectives):
  A: projections r1,i1,i2,r2 = x @ A-planes, single fp16 matmul each (rel-err
     budget 2e-2 allows it; measured ~1.2e-3 end to end)
  B: S[k,b] = sum_t P1*P2: ScalarE drains PSUM planes to SBUF fp16, then DVE
     tensor_tensor_reduce chains (fused multiply+reduce, fp32 accum) produce
     S_re = sum r1r2 - i1i2 and S_im = sum r1i2 + i1r2 directly
  C: IFFT via two-step factorization p=64q+s: Gauss 3-mult twiddle
     t1=cphi*S_re, t2=sphi*S_im, t3=(cphi+sphi)*(S_re+S_im), then
     psy = (cosa-nsina)@t1 - (cosa+nsina)@t2 + nsina@t3  (128-DFT matmuls)
  D: signed sqrt + per-batch L2 norm + store
"""
import numpy as np

P = 8192
C = 512
FT = 33            # frequency tiles of 128 -> 4224 slots >= 4097
NSLOT = FT * 128
NCORES = 8
BPC = 4            # batch elems per core
HW = 196           # spatial positions per batch elem
T = BPC * HW       # 784 positions per core
B = 32

_CACHE = {}


def _build_program():
    import concourse.bass as bass
    import concourse.tile as tile
    from concourse import bacc, mybir

    f32 = mybir.dt.float32
    f16 = mybir.dt.float16
    nc = bacc.Bacc("TRN2", target_bir_lowering=False, debug=False,
                   num_devices=NCORES)

    a_d = nc.dram_tensor("a", [FT, C, 512], f16, kind="ExternalInput").ap()
    x_d = nc.dram_tensor("x", [C, T], f16, kind="ExternalInput").ap()
    cphi_d = nc.dram_tensor("cphi", [FT, 128, 64], f32, kind="ExternalInput").ap()
    sphi_d = nc.dram_tensor("sphi", [FT, 128, 64], f32, kind="ExternalInput").ap()
    cpsphi_d = nc.dram_tensor("cpsphi", [FT, 128, 64], f32,
                              kind="ExternalInput").ap()
    mh_d = nc.dram_tensor("mh", [3, 128, 128], f16, kind="ExternalInput").ap()
    y_d = nc.dram_tensor("y", [BPC, P], f32, kind="ExternalOutput").ap()

    mult = mybir.AluOpType.mult
    add = mybir.AluOpType.add
    Act = mybir.ActivationFunctionType

    # plane m -> (psum pair tile, column offset); pairs are
    # T1 = [r1 | i1], T2 = [i2 | r2] (i2 first so the S_im pair-TTR reads
    # (r1,i1)x(i2,r2) with plain slices)
    with tile.TileContext(nc) as tc:
        with (
            tc.tile_pool(name="const", bufs=1) as const,
            tc.tile_pool(name="apool", bufs=3) as apool,
            tc.tile_pool(name="ps", bufs=3, space="PSUM") as pspool,
            tc.tile_pool(name="ps1", bufs=1, space="PSUM") as pspool1,
            tc.tile_pool(name="stg", bufs=2) as stg,
            tc.tile_pool(name="scr", bufs=3) as scr,
        ):
            # emission order = DMA queue order: first matmul needs x[ck0] + a0,
            # so those go first, bulky twiddle tables last
            x_src = x_d.rearrange("(ck p) t -> p ck t", p=128)
            x_sb = const.tile([128, 4, T], f16)
            nc.sync.dma_start(x_sb[:, 0], x_src[:, 0])
            a_tiles = {}
            for ft in range(3):
                a_t = apool.tile([128, 4, 512], f16, tag="a", name=f"a_{ft}")
                nc.sync.dma_start(
                    a_t[:], a_d[ft].rearrange("(ck p) m -> p ck m", p=128)
                )
                a_tiles[ft] = a_t
                if ft == 0:
                    nc.sync.dma_start(x_sb[:, 1:4], x_src[:, 1:4])
            cphi_sb = const.tile([128, FT, 64], f32)
            nc.sync.dma_start(cphi_sb[:], cphi_d.rearrange("kt p s -> p kt s"))
            sphi_sb = const.tile([128, FT, 64], f32)
            nc.sync.dma_start(sphi_sb[:], sphi_d.rearrange("kt p s -> p kt s"))
            cpsphi_sb = const.tile([128, FT, 64], f32)
            nc.sync.dma_start(cpsphi_sb[:], cpsphi_d.rearrange("kt p s -> p kt s"))
            mh_sb = const.tile([128, 3, 128], f16)
            nc.sync.dma_start(mh_sb[:], mh_d.rearrange("m p q -> p m q"))
            ones_sb = const.tile([128, 1], f32)
            nc.vector.memset(ones_sb[:], 1.0)
            # warm the ACT table set (sqrt anchor; abs/sign/copy ride along)
            warm = scr.tile([1, 1], f32, tag="warm")
            nc.vector.memset(warm[:], 1.0)
            nc.scalar.activation(warm[:], warm[:], Act.Sqrt)
            sre_sb = const.tile([128, FT * 4], f32)
            sim_sb = const.tile([128, FT * 4], f32)
            ssum_sb = const.tile([128, FT * 4], f32)
            accA_sb = const.tile([128, FT * 4], f32)
            accB_sb = const.tile([128, FT * 4], f32)
            tw_sb = const.tile([128, FT, 3, 4, 64], f16)

            # ---- stage A+B+twiddle, pipelined per frequency tile ----
            # per-plane PSUM tiles (2 banks each) rotate through 3 buffers;
            # psy owns its own bank so the stage-C IFFT matmuls interleave
            # with the loop (PSUM accumulation groups are bank-granular)
            psy = pspool1.tile([128, 512], f32, tag="psy")

            def stage_c(kt):
                for mat in range(3):
                    nc.tensor.matmul(
                        psy[:, 0:BPC * 64],
                        mh_sb[:, mat],
                        tw_sb[:, kt, mat].rearrange("p b s -> p (b s)"),
                        start=(mat == 0 and kt == 0),
                        stop=(mat == 2 and kt == FT - 1),
                    )

            for ft in range(FT):
                if ft in a_tiles:
                    a_t = a_tiles[ft]
                else:
                    a_t = apool.tile([128, 4, 512], f16, tag="a", name=f"a_{ft}")
                    nc.sync.dma_start(
                        a_t[:], a_d[ft].rearrange("(ck p) m -> p ck m", p=128)
                    )
                # m: 0->r1(t1sb:0) 1->i1(t1sb:1) 2->i2(t2sb:0) 3->r2(t2sb:1)
                # t2sb planes first so their drains overlap the t1 matmuls
                t2sb = stg.tile([128, 2, T], f16, tag="t2sb")
                t1sb = stg.tile([128, 2, T], f16, tag="t1sb")
                for m, sbt, half in ((2, t2sb, 0), (3, t2sb, 1),
                                     (0, t1sb, 0), (1, t1sb, 1)):
                    msl = slice(m * 128, (m + 1) * 128)
                    pt = pspool.tile([128, 1024], f32, tag="pl",
                                     name=f"pl_{ft}_{m}")
                    for ck in range(4):
                        for c0, cn in ((0, 512), (512, T - 512)):
                            nc.tensor.matmul(
                                pt[:, c0:c0 + cn],
                                a_t[:, ck, msl],
                                x_sb[:, ck, c0:c0 + cn],
                                start=(ck == 0),
                                stop=(ck == 3),
                            )
                    nc.scalar.activation(sbt[:, half], pt[:, 0:T], Act.Copy)
                # fused multiply+reduce (STT): S_re parts and S_im per batch elem
                for b in range(BPC):
                    bs = slice(b * HW, (b + 1) * HW)
                    slot = slice(ft * 4 + b, ft * 4 + b + 1)
                    po = scr.tile([128, HW], f16, tag=f"po{b}")
                    pp = scr.tile([128, 2, HW], f16, tag=f"pp{b}")
                    nc.vector.scalar_tensor_tensor(
                        out=po[:], in0=t1sb[:, 0, bs], scalar=1.0,
                        in1=t2sb[:, 1, bs], op0=mult, op1=mult,
                        accum_out=accA_sb[:, slot],
                    )
                    nc.vector.scalar_tensor_tensor(
                        out=po[:], in0=t1sb[:, 1, bs], scalar=1.0,
                        in1=t2sb[:, 0, bs], op0=mult, op1=mult,
                        accum_out=accB_sb[:, slot],
                    )
                    nc.vector.scalar_tensor_tensor(
                        out=pp[:], in0=t1sb[:, :, bs], scalar=1.0,
                        in1=t2sb[:, :, bs], op0=mult, op1=mult,
                        accum_out=sim_sb[:, slot],
                    )
                # Gauss twiddle products for this kt
                sl4 = slice(ft * 4, (ft + 1) * 4)
                nc.vector.tensor_sub(sre_sb[:, sl4], accA_sb[:, sl4],
                                     accB_sb[:, sl4])
                nc.vector.tensor_add(ssum_sb[:, sl4], sre_sb[:, sl4],
                                     sim_sb[:, sl4])
                for mat, tbl, s_in in ((0, cphi_sb, sre_sb),
                                       (1, sphi_sb, sim_sb),
                                       (2, cpsphi_sb, ssum_sb)):
                    nc.vector.tensor_tensor(
                        tw_sb[:, ft, mat],
                        tbl[:, ft, :][:, None, :].broadcast_to([128, BPC, 64]),
                        s_in[:, sl4][:, :, None].broadcast_to([128, BPC, 64]),
                        op=mult,
                    )

            # ---- stage C: IFFT matmuls, kt-ordered so each fires when its
            # twiddle lands; psy bank is free of other accumulation groups
            for kt in range(FT):
                stage_c(kt)

            # ---- stage D: signed sqrt, per-batch l2 norm, store ----
            absy = scr.tile([128, BPC * 64], f32, tag="absy")
            sgn = scr.tile([128, BPC * 64], f32, tag="sgn")
            nc.scalar.activation(absy[:], psy[:, 0:BPC * 64], Act.Abs)
            nc.scalar.activation(sgn[:], psy[:, 0:BPC * 64], Act.Sign)
            sqy = scr.tile([128, BPC * 64], f32, tag="sqy")
            nc.scalar.activation(sqy[:], absy[:], Act.Sqrt)
            ys = scr.tile([128, BPC * 64], f32, tag="ys")
            nc.vector.tensor_mul(ys[:], sqy[:], sgn[:])

            psnt = pspool1.tile([128, 512], f32, tag="psn")
            psn = psnt[0:1, 0:BPC * 64]
            nc.tensor.matmul(psn, ones_sb[:], absy[:],
                             start=True, stop=True)
            nsq = scr.tile([1, BPC], f32, tag="nsq")
            nc.vector.reduce_sum(
                out=nsq[:],
                in_=psn.rearrange("p (b s) -> p b s", b=BPC),
                axis=mybir.AxisListType.X,
            )
            nc.vector.tensor_scalar_max(nsq[:], nsq[:], 1e-10)
            sqn = scr.tile([1, BPC], f32, tag="sqn")
            nc.scalar.activation(sqn[:], nsq[:], Act.Sqrt)
            invn = scr.tile([1, BPC], f32, tag="invn")
            nc.vector.reciprocal(invn[:], sqn[:])

            onesrow = const.tile([1, 128], f32)
            nc.vector.memset(onesrow[:], 1.0)
            psb = psnt[:, 256:256 + BPC]
            nc.tensor.matmul(psb, onesrow[0:1, :], invn[0:1, :],
                             start=True, stop=True)
            inv_b = psb[:, :, None].broadcast_to([128, BPC, 64])
            fin = scr.tile([128, BPC * 64], f32, tag="fin")
            nc.vector.tensor_tensor(
                fin[:].rearrange("p (b s) -> p b s", b=BPC),
                ys[:].rearrange("p (b s) -> p b s", b=BPC),
                inv_b,
                op=mult,
            )
            nc.sync.dma_start(
                y_d.rearrange("b (q s) -> q b s", q=128),
                fin[:].rearrange("p (b s) -> p b s", b=BPC),
            )

    nc.compile()
    return nc


def _host_prep(x, M1, M2):
    x = np.ascontiguousarray(np.asarray(x, np.float32))
    M1 = np.asarray(M1, np.float32)
    M2 = np.asarray(M2, np.float32)

    h1 = np.argmax(np.abs(M1), axis=1)
    s1 = M1[np.arange(C), h1].astype(np.float64)
    h2 = np.argmax(np.abs(M2), axis=1)
    s2 = M2[np.arange(C), h2].astype(np.float64)

    k = np.arange(NSLOT, dtype=np.float64)
    valid = k <= P // 2
    ang1 = 2 * np.pi * np.outer(h1.astype(np.float64), k) / P
    ang2 = 2 * np.pi * np.outer(h2.astype(np.float64), k) / P
    # a[ft, c, m*128 + j]: m planes (A1re, A1im, A2im, A2re), freq = ft*128 + j
    a = np.empty((FT, C, 512), np.float16)
    a1re = (s1[:, None] * np.cos(ang1) * valid).astype(np.float16)
    a1im = (-s1[:, None] * np.sin(ang1) * valid).astype(np.float16)
    a2re = (s2[:, None] * np.cos(ang2) * valid).astype(np.float16)
    a2im = (-s2[:, None] * np.sin(ang2) * valid).astype(np.float16)
    for ft in range(FT):
        ksl = slice(ft * 128, (ft + 1) * 128)
        a[ft, :, 0:128] = a1re[:, ksl]
        a[ft, :, 128:256] = a1im[:, ksl]
        a[ft, :, 256:384] = a2im[:, ksl]
        a[ft, :, 384:512] = a2re[:, ksl]

    w = np.where(valid, 2.0 / P, 0.0)
    w[0] = 1.0 / P
    w[P // 2] = 1.0 / P
    s_idx = np.arange(64, dtype=np.float64)
    phi = 2 * np.pi * np.outer(k, s_idx) / P
    cphi = (w[:, None] * np.cos(phi)).astype(np.float32).reshape(FT, 128, 64)
    sphi = (w[:, None] * np.sin(phi)).astype(np.float32).reshape(FT, 128, 64)
    cpsphi = (w[:, None] * (np.cos(phi) + np.sin(phi))).astype(
        np.float32).reshape(FT, 128, 64)

    km = np.arange(128, dtype=np.float64)
    alpha = 2 * np.pi * np.outer(km, km) / 128
    mh = np.empty((3, 128, 128), np.float16)
    mh[0] = (np.cos(alpha) + np.sin(alpha)).astype(np.float16)   # cosa - nsina
    mh[1] = (np.sin(alpha) - np.cos(alpha)).astype(np.float16)   # -(cosa+nsina)
    mh[2] = (-np.sin(alpha)).astype(np.float16)                  # nsina

    xt = np.ascontiguousarray(x.reshape(B * HW, C).T).astype(np.float16)

    return a, cphi, sphi, cpsphi, mh, xt


def _make_in_maps(x, M1, M2):
    a, cphi, sphi, cpsphi, mh, xt = _host_prep(x, M1, M2)
    in_maps = []
    for r in range(NCORES):
        in_maps.append({
            "a": a,
            "x": np.ascontiguousarray(xt[:, r * T:(r + 1) * T]),
            "cphi": cphi,
            "sphi": sphi,
            "cpsphi": cpsphi,
            "mh": mh,
        })
    return in_maps


def kernel(x, M1, M2):
    from concourse.bass_utils import run_bass_kernel_spmd

    if "nc" not in _CACHE:
        _CACHE["nc"] = _build_program()
    nc = _CACHE["nc"]

    in_maps = _make_in_maps(x, M1, M2)
    res = run_bass_kernel_spmd(nc, in_maps, core_ids=list(range(NCORES)))
    out = np.concatenate([res.results[r]["y"] for r in range(NCORES)], axis=0)
    return out.astype(np.float32)
